# revision 25
# baseline (speedup 1.0000x reference)
"""Trainium2 Bass kernel for the 21x21 correlation (cost volume) module.

Math: out[b, di*21+dj, i, j] = sum_c x1p[b, c, i+di, j+dj] * x2[b, c, i, j]
where x1p is x1 zero-padded by 10 on both spatial dims, di,dj in [0,21).

Strategy (8 NeuronCores, SPMD, no collectives):
  - Shard: batch (4) x W-halves (2). Core k -> (b = k//2, rows i in
    [64*(k%2), 64*(k%2)+64)). Inputs shipped as fp16 (host cast; the
    2e-2 rel-err budget dwarfs fp16 quantization).
  - On-core: channels C=128 on the SBUF partition dim (= matmul K).
    Patches of 16x8 pixels (pi-major partition order p = pi*8+pj); the
    36x28 x1 window streams STRAIGHT from the resident x1 tile via a
    strided 3-dim rhs AP (no repack). Two matmuls per patch (N=504 =
    18x28 window halves) write one 2-bank PSUM tile at elem offsets
    8 and 512, so each half stays inside a 2KB bank yet the pair is
    contiguous at [8:1016] for a single evacuation copy.
  - Evacuation: one whole-patch copy per engine (DVE takes even
    patches, Act odd ones), fp32 -> fp16 cast in the copy. GpSimd
    cannot read PSUM on TRN2, so these are the only two lanes.
  - Output DMA per (band, pi-pair) extracts only window rows
    2k..2k+21 (22x28 = 616 of 1008 per pixel, 1.40x inflation vs the
    dense 2.29x) with 1232-byte runs; all DMAs stay on the SP/HWDGE
    path (SWDGE and >~48 total DMAs both regress: 625 ns issue each
    on the shared HWDGE device). The last band runs as two half-band
    tiles shipping pi-QUADS (rows 4k..4k+23, 672/pixel) so its DMAs
    gate on half-band evacuation and the post-compute tail halves.
    Host de-shears the (di,dj) band with as_strided for free and
    casts back to fp32.
  - Input DMAs are chunked (x2's first two patch columns, then x1 in
    18/16-row chunks, then the rest per band) so the first matmul
    starts after ~4 us of input traffic instead of all 14.7 us.

Cost-model notes (TimelineSim, the graded metric): all DMAs serialize
on one DMA_ENGINES device at 360 GB/s aggregate (descriptor = one
per-partition run; runs under 512B pay 2x); matmul costs out-free-size
x 0.4167 ns regardless of K/M; DVE/Act engine copies cost ~1.04/0.83
ns per free element. Per core this kernel moves 5.3 MB in + 10.3 MB
out (~42.7 us DMA floor) and paces evacuation at ~590 ns/patch.
"""
import sys

if "/opt/trn_rl_repo" not in sys.path:
    sys.path.insert(0, "/opt/trn_rl_repo")

import numpy as np
from numpy.lib.stride_tricks import as_strided

import concourse.bass as bass
import concourse.mybir as mybir
import concourse.tile as tile
from concourse import bacc
from concourse.bass_utils import run_bass_kernel_spmd

B, C, W, H = 4, 128, 128, 128
DW = 21          # displacement window (per axis)
PAD = 10
N_CORES = 8
PI, PJ = 16, 8           # patch shape (pixels); partition p = pi*8 + pj
IB, JB = 4, 16           # patch grid per core (4 row-bands x 16 col-patches)
RW, QW = PI + DW - 1, PJ + DW - 1    # streamed window 36 x 28
NSTREAM = RW * QW        # 1008
NPAIR = PI // 2          # 8 pi-pairs per band
ROWS_PAIR = DW + 1       # 22 window rows cover a pi-pair
EPP = ROWS_PAIR * QW     # 616 elements written per pixel (pair DMAs)
EPQ = (DW + 3) * QW      # 672: 24 window rows cover a pi-quad
HALO_ROWS = 64 + 2 * PAD     # 84
PADDED_COLS = H + 2 * PAD    # 148

F16 = mybir.dt.float16
F32 = mybir.dt.float32

_CACHE = {}


def _build_program():
    nc = bacc.Bacc("TRN2", target_bir_lowering=False, debug=False,
                   num_devices=N_CORES)
    x1h = nc.dram_tensor("x1h", [C, HALO_ROWS, PADDED_COLS], F16,
                         kind="ExternalInput")
    # x2 shipped patch-major: [c, ib, jb, p] with p = pi*8 + pj.
    x2s = nc.dram_tensor("x2s", [C, IB, JB, PI * PJ], F16,
                         kind="ExternalInput")
    # Bands 0..IB-2 ship as pi-pairs; the last band ships as pi-quads
    # from half-band tiles (fewer, earlier-gated DMAs in the tail).
    outp = nc.dram_tensor("outp", [IB - 1, NPAIR, 16, JB, EPP], F16,
                          kind="ExternalOutput")
    outq = nc.dram_tensor("outq", [2, 4, 32, JB // 2, EPQ], F16,
                          kind="ExternalOutput")

    with tile.TileContext(nc) as tc:
        with (
            tc.tile_pool(name="singles", bufs=1) as singles,
            tc.tile_pool(name="outs", bufs=3) as outs,
            tc.tile_pool(name="psum", bufs=2, space="PSUM") as psum,
        ):
            x1_sb = singles.tile([C, HALO_ROWS, PADDED_COLS], F16)
            x2_sb = singles.tile([C, IB, JB, PI * PJ], F16)
            # Chunked loads, finest pieces first, so band 0's first
            # patches start compute almost immediately.
            nc.sync.dma_start(out=x2_sb[:, 0, 0:2], in_=x2s[:, 0, 0:2])
            nc.sync.dma_start(out=x1_sb[:, 0:18], in_=x1h[:, 0:18])
            nc.sync.dma_start(out=x1_sb[:, 18:36], in_=x1h[:, 18:36])
            nc.sync.dma_start(out=x2_sb[:, 0, 2:16], in_=x2s[:, 0, 2:16])
            for ib in range(1, IB):
                r0, r1 = ib * 16 + 20, min(ib * 16 + 36, HALO_ROWS)
                nc.sync.dma_start(out=x1_sb[:, r0:r1], in_=x1h[:, r0:r1])
                nc.sync.dma_start(out=x2_sb[:, ib], in_=x2s[:, ib])

            def do_patch(ib, jb, ps):
                lhsT = x2_sb[:, ib, jb, :]
                win = x1_sb[:, ib * PI:ib * PI + RW,
                            jb * PJ:jb * PJ + QW]
                nc.tensor.matmul(ps[:, 8:512], lhsT=lhsT,
                                 rhs=win[:, 0:18, :], start=True, stop=True)
                nc.tensor.matmul(ps[:, 512:1016], lhsT=lhsT,
                                 rhs=win[:, 18:36, :], start=True, stop=True)

            def compute(ib, ot, jb_lo, jb_hi):
                for jp in range(jb_lo // 2, jb_hi // 2):
                    jb0, jb1 = 2 * jp, 2 * jp + 1
                    pa = psum.tile([128, 1024], F32, name="pa")
                    pb = psum.tile([128, 1024], F32, name="pb")
                    do_patch(ib, jb0, pa)
                    do_patch(ib, jb1, pb)
                    # One whole-patch evacuation copy per engine (the
                    # only two engines that can read PSUM).
                    nc.vector.tensor_copy(ot[:, jb0 - jb_lo, :],
                                          pa[:, 8:1016])
                    nc.scalar.copy(out=ot[:, jb1 - jb_lo, :],
                                   in_=pb[:, 8:1016])

            # Full-band tiles with pair-granular DMAs for bands 0..IB-2;
            # the last band runs in half-band tiles shipping pi-quads,
            # so its (fewer) DMAs start at the half-band mark instead of
            # all queuing after the final evacuation.
            for ib in range(IB - 1):
                ot = outs.tile([128, JB, NSTREAM], F16)
                compute(ib, ot, 0, JB)
                for k in range(NPAIR):
                    # pi-pair {2k, 2k+1} = partitions [16k, 16k+16);
                    # window rows 2k..2k+21 -> elems [56k, 56k+616).
                    nc.sync.dma_start(
                        out=outp[ib, k],
                        in_=ot[16 * k:16 * k + 16, :,
                               56 * k:56 * k + EPP])
            for h in range(2):
                oth = outs.tile([128, JB // 2, NSTREAM], F16, name="oth")
                compute(IB - 1, oth, h * (JB // 2), (h + 1) * (JB // 2))
                for k in range(4):
                    # pi-quad {4k..4k+3} = partitions [32k, 32k+32);
                    # window rows 4k..4k+23 -> elems [112k, 112k+672).
                    nc.sync.dma_start(
                        out=outq[h, k],
                        in_=oth[32 * k:32 * k + 32, :,
                                112 * k:112 * k + EPQ])

    nc.finalize()
    return nc


def _shard_inputs(x1, x2):
    in_maps = []
    for k in range(N_CORES):
        b, half = divmod(k, 2)
        i0 = 64 * half
        x2sh = np.ascontiguousarray(
            x2[b][:, i0:i0 + 64, :]
            .reshape(C, IB, PI, JB, PJ)
            .transpose(0, 1, 3, 2, 4)
            .reshape(C, IB, JB, PI * PJ)
        ).astype(np.float16)
        x1sh = np.zeros((C, HALO_ROWS, PADDED_COLS), np.float16)
        rlo, rhi = i0 - PAD, i0 + 64 + PAD
        slo, shi = max(rlo, 0), min(rhi, W)
        x1sh[:, slo - rlo:shi - rlo, PAD:PAD + H] = \
            x1[b][:, slo:shi, :].astype(np.float16)
        in_maps.append({"x1h": x1sh, "x2s": x2sh})
    return in_maps


def _gather(results):
    out = np.empty((B, DW * DW, W, H), np.float32)
    for k in range(N_CORES):
        b, half = divmod(k, 2)
        i0 = 64 * half
        # Bands 0..IB-2 from pair staging [IB-1, 8, 16, JB, 616]:
        # O[ib, pair, pil*8+pj, jb, (pil+di)*28 + pj+dj]
        O = np.ascontiguousarray(results[k]["outp"])
        e = O.itemsize
        s_ib, s_pair, s_part, s_jb = (np.array(O.strides[:4]) // e)
        sv = as_strided(
            O,
            shape=(IB - 1, NPAIR, 2, PJ, JB, DW, DW),
            strides=tuple(np.array(
                [s_ib, s_pair, 8 * s_part + QW, s_part + 1, s_jb, QW, 1]
            ) * e),
        )
        out[b, :, i0:i0 + 48, :] = (
            sv.transpose(5, 6, 0, 1, 2, 4, 3)
            .reshape(DW * DW, 48, H)
            .astype(np.float32)
        )
        # Last band from quad staging [2, 4, 32, JB/2, 672]:
        # Q[h, quad, pil*8+pj, jbh, (pil+di)*28 + pj+dj], pi = 4*quad+pil
        Q = np.ascontiguousarray(results[k]["outq"])
        e = Q.itemsize
        q_h, q_quad, q_part, q_jb = (np.array(Q.strides[:4]) // e)
        qv = as_strided(
            Q,
            shape=(2, 4, 4, PJ, JB // 2, DW, DW),
            strides=tuple(np.array(
                [q_h, q_quad, 8 * q_part + QW, q_part + 1, q_jb, QW, 1]
            ) * e),
        )
        # axes -> [di, dj, quad, pil, h, jbh, pj] -> [441, 16, 128]
        out[b, :, i0 + 48:i0 + 64, :] = (
            qv.transpose(5, 6, 1, 2, 0, 4, 3)
            .reshape(DW * DW, 16, H)
            .astype(np.float32)
        )
    return out


def kernel(x1, x2):
    x1 = np.asarray(x1, dtype=np.float32)
    x2 = np.asarray(x2, dtype=np.float32)
    if "nc" not in _CACHE:
        _CACHE["nc"] = _build_program()
    nc = _CACHE["nc"]
    in_maps = _shard_inputs(x1, x2)
    res = run_bass_kernel_spmd(nc, in_maps, list(range(N_CORES)))
    return _gather(res.results)


# revision 26
# speedup vs baseline: 1.0645x; 1.0645x over previous
"""Trainium2 Bass kernel for the 21x21 correlation (cost volume) module.

Math: out[b, di*21+dj, i, j] = sum_c x1p[b, c, i+di, j+dj] * x2[b, c, i, j]
where x1p is x1 zero-padded by 10 on both spatial dims, di,dj in [0,21).

Strategy (8 NeuronCores, SPMD, no collectives):
  - Shard: batch (4) x W-halves (2). Core k -> (b = k//2, rows i in
    [64*(k%2), 64*(k%2)+64)). Inputs shipped as fp16 (host cast; the
    2e-2 rel-err budget dwarfs fp16 quantization).
  - On-core: channels C=128 on the SBUF partition dim (= matmul K).
    Patches of 16x8 pixels (pi-major partition order p = pi*8+pj); the
    36x28 x1 window streams STRAIGHT from the resident x1 tile via a
    strided 3-dim rhs AP (no repack). Two matmuls per patch (N=504 =
    18x28 window halves) write one 2-bank PSUM tile at elem offsets
    8 and 512, so each half stays inside a 2KB bank yet the pair is
    contiguous at [8:1016] for a single evacuation copy.
  - Evacuation: one whole-patch copy, alternating DVE / Act per patch
    (the only two engines that can read PSUM; GpSimd cannot on TRN2).
    PSUM rotates per patch (bufs=4 of one 2-bank tile) for the finest
    matmul->copy pipelining. Three extra patches go to Act (the faster
    engine) to balance total evacuation time across the two lanes.
  - Warm start: a small host-packed duplicate of the first 6 patch
    windows (x1f, [C,36,68], contiguous 0.7us DMA) lets the evacuation
    chain start at ~4.5us instead of waiting ~7us for the full-width
    x1 row chunks.
  - Output DMA extracts only the useful window-row bands per partition
    group: bands 0-1 ship pi-PAIRS (rows 2k..2k+21, 616 of 1008 per
    pixel, 1232-byte runs) from full-band tiles; bands 2-3 ship
    pi-QUADS (rows 4k..4k+23, 672/pixel) from half-band tiles so the
    final DMAs gate at half-band marks and the post-compute tail
    shrinks. All DMAs stay on SP/HWDGE (SWDGE and >~48 DMAs regress).
    Host de-shears the (di,dj) band with as_strided for free and
    casts back to fp32.
  - Input DMAs are chunked so the first matmul starts after ~2.7us of
    input traffic instead of all ~15us.

Cost-model notes (TimelineSim, the graded metric): all DMAs serialize
on one DMA_ENGINES device at 360 GB/s aggregate (descriptor = one
per-partition run; runs under 512B pay 2x); matmul costs out-free-size
x 0.4167 ns regardless of K/M; DVE/Act engine copies cost ~1.04/0.83
ns per free element. Per core this kernel moves 5.4 MB in + 10.4 MB
out (~44 us DMA floor) and paces evacuation at ~560 ns/patch.
"""
import sys

if "/opt/trn_rl_repo" not in sys.path:
    sys.path.insert(0, "/opt/trn_rl_repo")

import numpy as np
from numpy.lib.stride_tricks import as_strided

import concourse.bass as bass
import concourse.mybir as mybir
import concourse.tile as tile
from concourse import bacc
from concourse.bass_utils import run_bass_kernel_spmd

B, C, W, H = 4, 128, 128, 128
DW = 21          # displacement window (per axis)
PAD = 10
N_CORES = 8
PI, PJ = 16, 8           # patch shape (pixels); partition p = pi*8 + pj
IB, JB = 4, 16           # patch grid per core (4 row-bands x 16 col-patches)
RW, QW = PI + DW - 1, PJ + DW - 1    # streamed window 36 x 28
NSTREAM = RW * QW        # 1008
NPAIR = PI // 2          # 8 pi-pairs per band
EPP = (DW + 1) * QW      # 616: 22 window rows cover a pi-pair
EPQ = (DW + 3) * QW      # 672: 24 window rows cover a pi-quad
NWARM = 6                # band-0 patches served from the warm tile
WARM_COLS = 20 + 8 * NWARM   # 68
HALO_ROWS = 64 + 2 * PAD     # 84
PADDED_COLS = H + 2 * PAD    # 148
ACT_BOTH = (10, 32, 54)  # global patch indices Act takes from DVE

F16 = mybir.dt.float16
F32 = mybir.dt.float32

_CACHE = {}


def _build_program():
    nc = bacc.Bacc("TRN2", target_bir_lowering=False, debug=False,
                   num_devices=N_CORES)
    x1h = nc.dram_tensor("x1h", [C, HALO_ROWS, PADDED_COLS], F16,
                         kind="ExternalInput")
    x1f = nc.dram_tensor("x1f", [C, RW, WARM_COLS], F16,
                         kind="ExternalInput")
    # x2 shipped patch-major: [c, ib, jb, p] with p = pi*8 + pj.
    x2s = nc.dram_tensor("x2s", [C, IB, JB, PI * PJ], F16,
                         kind="ExternalInput")
    # Bands 0-1 ship as pi-pairs, bands 2-3 as pi-quads.
    outp = nc.dram_tensor("outp", [2, NPAIR, 16, JB, EPP], F16,
                          kind="ExternalOutput")
    outq = nc.dram_tensor("outq", [2, 4, 32, JB, EPQ], F16,
                          kind="ExternalOutput")

    with tile.TileContext(nc) as tc:
        with (
            tc.tile_pool(name="singles", bufs=1) as singles,
            tc.tile_pool(name="outs", bufs=3) as outs,
            tc.tile_pool(name="psum", bufs=4, space="PSUM") as psum,
        ):
            x1_sb = singles.tile([C, HALO_ROWS, PADDED_COLS], F16)
            x1f_sb = singles.tile([C, RW, WARM_COLS], F16)
            x2_sb = singles.tile([C, IB, JB, PI * PJ], F16)
            # Chunked loads, finest pieces first: the warm tile plus the
            # first 6 x2 columns gate band 0's first patches at ~2.7us.
            nc.sync.dma_start(out=x2_sb[:, 0, 0:NWARM],
                              in_=x2s[:, 0, 0:NWARM])
            nc.sync.dma_start(out=x1f_sb, in_=x1f[:, :, :])
            nc.sync.dma_start(out=x1_sb[:, 0:18], in_=x1h[:, 0:18])
            nc.sync.dma_start(out=x1_sb[:, 18:36], in_=x1h[:, 18:36])
            nc.sync.dma_start(out=x2_sb[:, 0, NWARM:16],
                              in_=x2s[:, 0, NWARM:16])
            for ib in range(1, IB):
                r0, r1 = ib * 16 + 20, min(ib * 16 + 36, HALO_ROWS)
                nc.sync.dma_start(out=x1_sb[:, r0:r1], in_=x1h[:, r0:r1])
                nc.sync.dma_start(out=x2_sb[:, ib], in_=x2s[:, ib])

            gpatch = [0]

            def do_patch(ib, jb, ps):
                lhsT = x2_sb[:, ib, jb, :]
                if ib == 0 and jb < NWARM:
                    win = x1f_sb[:, :, jb * PJ:jb * PJ + QW]
                else:
                    win = x1_sb[:, ib * PI:ib * PI + RW,
                                jb * PJ:jb * PJ + QW]
                nc.tensor.matmul(ps[:, 8:512], lhsT=lhsT,
                                 rhs=win[:, 0:18, :], start=True, stop=True)
                nc.tensor.matmul(ps[:, 512:1016], lhsT=lhsT,
                                 rhs=win[:, 18:36, :], start=True, stop=True)

            def compute(ib, ot, jb_lo, jb_hi):
                for jb in range(jb_lo, jb_hi):
                    ps = psum.tile([128, 1024], F32, name="pc")
                    do_patch(ib, jb, ps)
                    g = gpatch[0]
                    if (g % 2 == 1) or (g in ACT_BOTH):
                        nc.scalar.copy(out=ot[:, jb - jb_lo, :],
                                       in_=ps[:, 8:1016])
                    else:
                        nc.vector.tensor_copy(ot[:, jb - jb_lo, :],
                                              ps[:, 8:1016])
                    gpatch[0] += 1

            for ib in range(2):
                ot = outs.tile([128, JB, NSTREAM], F16, name="ot16")
                compute(ib, ot, 0, JB)
                for k in range(NPAIR):
                    # pi-pair {2k, 2k+1} = partitions [16k, 16k+16);
                    # window rows 2k..2k+21 -> elems [56k, 56k+616).
                    nc.sync.dma_start(
                        out=outp[ib, k],
                        in_=ot[16 * k:16 * k + 16, :,
                               56 * k:56 * k + EPP])
            for ib in range(2, IB):
                for h in range(2):
                    oth = outs.tile([128, JB // 2, NSTREAM], F16, name="ot8")
                    lo = h * (JB // 2)
                    compute(ib, oth, lo, lo + JB // 2)
                    for k in range(4):
                        # pi-quad {4k..4k+3} = partitions [32k, 32k+32);
                        # window rows 4k..4k+23 -> elems [112k, 112k+672).
                        nc.sync.dma_start(
                            out=outq[ib - 2, k, :, lo:lo + JB // 2],
                            in_=oth[32 * k:32 * k + 32, :,
                                    112 * k:112 * k + EPQ])

    nc.finalize()
    return nc


def _shard_inputs(x1, x2):
    in_maps = []
    for k in range(N_CORES):
        b, half = divmod(k, 2)
        i0 = 64 * half
        x2sh = np.ascontiguousarray(
            x2[b][:, i0:i0 + 64, :]
            .reshape(C, IB, PI, JB, PJ)
            .transpose(0, 1, 3, 2, 4)
            .reshape(C, IB, JB, PI * PJ)
        ).astype(np.float16)
        x1sh = np.zeros((C, HALO_ROWS, PADDED_COLS), np.float16)
        rlo, rhi = i0 - PAD, i0 + 64 + PAD
        slo, shi = max(rlo, 0), min(rhi, W)
        x1sh[:, slo - rlo:shi - rlo, PAD:PAD + H] = \
            x1[b][:, slo:shi, :].astype(np.float16)
        x1fsh = np.ascontiguousarray(x1sh[:, 0:RW, 0:WARM_COLS])
        in_maps.append({"x1h": x1sh, "x1f": x1fsh, "x2s": x2sh})
    return in_maps


def _gather(results):
    out = np.empty((B, DW * DW, W, H), np.float32)
    for k in range(N_CORES):
        b, half = divmod(k, 2)
        i0 = 64 * half
        # Bands 0-1 from pair staging [2, 8, 16, JB, 616]:
        # O[ib, pair, pil*8+pj, jb, (pil+di)*28 + pj+dj]
        O = np.ascontiguousarray(results[k]["outp"])
        e = O.itemsize
        s_ib, s_pair, s_part, s_jb = (np.array(O.strides[:4]) // e)
        sv = as_strided(
            O,
            shape=(2, NPAIR, 2, PJ, JB, DW, DW),
            strides=tuple(np.array(
                [s_ib, s_pair, 8 * s_part + QW, s_part + 1, s_jb, QW, 1]
            ) * e),
        )
        out[b, :, i0:i0 + 32, :] = (
            sv.transpose(5, 6, 0, 1, 2, 4, 3)
            .reshape(DW * DW, 32, H)
            .astype(np.float32)
        )
        # Bands 2-3 from quad staging [2, 4, 32, JB, 672]:
        # Q[b2, quad, pil*8+pj, jb, (pil+di)*28 + pj+dj], pi = 4*quad+pil
        Q = np.ascontiguousarray(results[k]["outq"])
        e = Q.itemsize
        q_b2, q_quad, q_part, q_jb = (np.array(Q.strides[:4]) // e)
        qv = as_strided(
            Q,
            shape=(2, 4, 4, PJ, JB, DW, DW),
            strides=tuple(np.array(
                [q_b2, q_quad, 8 * q_part + QW, q_part + 1, q_jb, QW, 1]
            ) * e),
        )
        out[b, :, i0 + 32:i0 + 64, :] = (
            qv.transpose(5, 6, 0, 1, 2, 4, 3)
            .reshape(DW * DW, 32, H)
            .astype(np.float32)
        )
    return out


def kernel(x1, x2):
    x1 = np.asarray(x1, dtype=np.float32)
    x2 = np.asarray(x2, dtype=np.float32)
    if "nc" not in _CACHE:
        _CACHE["nc"] = _build_program()
    nc = _CACHE["nc"]
    in_maps = _shard_inputs(x1, x2)
    res = run_bass_kernel_spmd(nc, in_maps, list(range(N_CORES)))
    return _gather(res.results)
